# revision 41
# baseline (speedup 1.0000x reference)
"""Trainium2 Bass kernel for EvoAttn (B=2, L=2048, E=1024, H=16, D=64, causal,
multiplicative attention mask on q/k/v, fp32).

Sharding: batch*heads across 8 cores. Core c handles batch c//4, heads
[4*(c%4), 4*(c%4)+4). Each core computes its 4 heads' q/k/v projections
(column-parallel), full local attention, and a partial out-projection
(row-parallel). Partials are summed on the host (unshard) and bias added.

Layout notes (per core):
  xT   [1024, 2048]  = (x[b] * mask[b][:,None]).T           (host-prepared)
  wqT/wkT/wvT [1024, 256] = W[rows].T for the 256 local dims
  woM  [256, 1024]   = Wo[:, local].T
  qT/kT [256, 2048] in two partition blocks; head h at partitions 64*(h%2)..+63
  of block h//2. Scores are computed transposed (keys on partitions, queries on
  the free dim) so softmax needs no transposes: V is augmented with a ones
  column so the attn@V matmul also emits the softmax denominator. The V
  augmentation layout depends on head parity so each head's output lands on
  the partition half its yT slot needs (DVE ops cannot cross partitions):
    even head: [v(64) | ones(1) | zeros(63)] -> y rows 0..63,  denom row 64
    odd head:  [ones(1) | zeros(63) | v(64)] -> y rows 64..127, denom row 0
  Causal masking = skip fully-masked key blocks + add a -1e30 staircase mask
  (via an identity matmul) into partially-masked score blocks in PSUM.

Compute dtype config (KOMP env): "bf16" everything bf16; "mixed" keeps the
q/k/score path in float32r (tf32-like) for accuracy and uses bf16 for the
attn@V / out-proj bulk; "f32r" everything float32r.
"""

import os
import numpy as np

B, L, E, H, D = 2, 2048, 1024, 16, 64
DLOC = E // 4          # local out dims per core (4 heads * 64)
NEG = -1.0e30
KOMP = os.environ.get("KOMP", "bf16")

_CACHE = {}
LAST_RESULTS = None


def _dtype_cfg(mybir):
    f32r, bf16 = mybir.dt.float32r, mybir.dt.bfloat16
    if KOMP == "bf16":
        return dict(qk=bf16, v=bf16, y=bf16, mask=bf16)
    if KOMP == "mixed":
        return dict(qk=f32r, v=bf16, y=bf16, mask=f32r)
    return dict(qk=f32r, v=f32r, y=f32r, mask=f32r)


def _build_program():
    from contextlib import ExitStack

    import concourse.bacc as bacc
    import concourse.mybir as mybir
    import concourse.tile as tile

    f32 = mybir.dt.float32
    f32r = mybir.dt.float32r
    Exp = mybir.ActivationFunctionType.Exp
    dts = _dtype_cfg(mybir)
    dt_qk, dt_v, dt_y, dt_mask = dts["qk"], dts["v"], dts["y"], dts["mask"]
    # x and wq/wk feed the q/k path; wv feeds v (paired with x in its matmul,
    # so it must match x's dtype)
    dt_x = dt_qk

    nc = bacc.Bacc("TRN2", target_bir_lowering=False, debug=False, num_devices=8)

    def dram_in(name, shape, dt):
        # float32r inputs are declared f32 and bitcast (same bits); bf16
        # inputs are host-converted
        dd = f32 if dt in (f32, f32r) else dt
        ap = nc.dram_tensor(name, shape, dd, kind="ExternalInput").ap()
        return ap.bitcast(dt) if dt == f32r else ap

    xT = dram_in("xT", [E, L], dt_x)
    wqT = dram_in("wqT", [E, DLOC], dt_qk)
    wkT = dram_in("wkT", [E, DLOC], dt_qk)
    wvT = dram_in("wvT", [E, DLOC], dt_x)
    woM = dram_in("woM", [DLOC, E], dt_y)
    # 0/1 causal masks for the two partial chunk-pair offsets (see phase 2)
    pairmask = dram_in("pairmask", [128, 2048], dt_v)
    zpad = dram_in("zpad", [128, 512], f32r)
    vconst = dram_in("vconst", [128, 512], dt_v)
    # bc selector patterns: [:,0:64] row-64 extractor, [:,64:192] row-0 ->
    # output partitions 64..127 extractor
    bcsel = dram_in("bcsel", [128, 128], f32r)
    outp = nc.dram_tensor("outp", [L, E], f32, kind="ExternalOutput").ap()

    with (
        tile.TileContext(nc) as tc,
        ExitStack() as ctx,
        nc.allow_low_precision(reason="float32r/bf16 matmul inputs"),
    ):
        const_pool = ctx.enter_context(tc.tile_pool(name="const", bufs=1))
        w_pool = ctx.enter_context(tc.tile_pool(name="wp", bufs=1))
        qk_pool = ctx.enter_context(tc.tile_pool(name="qk", bufs=1))
        v_pool = ctx.enter_context(tc.tile_pool(name="vp", bufs=1))
        y_pool = ctx.enter_context(tc.tile_pool(name="yp", bufs=1))
        pp_psum = ctx.enter_context(tc.tile_pool(name="pp", bufs=2, space="PSUM"))
        sc_psum = ctx.enter_context(tc.tile_pool(name="sc", bufs=2, space="PSUM"))
        ya_psum = ctx.enter_context(tc.tile_pool(name="ya", bufs=2, space="PSUM"))

        # ---- constants (small loads, issued on scalar engine) ----
        pm_sb = const_pool.tile([128, 2048], dt_v, tag="pm")
        nc.scalar.dma_start(pm_sb, pairmask)
        bcsel_sb = const_pool.tile([128, 128], f32r, tag="bcsel")
        nc.scalar.dma_start(bcsel_sb, bcsel)

        q_sb = qk_pool.tile([128, 2, L], dt_qk, tag="q")
        k_sb = qk_pool.tile([128, 2, L], dt_qk, tag="k")
        # v augmented per head parity: 128 cols per head (see module docstring)
        v_sb = v_pool.tile([128, 16, 4 * 128], dt_v, tag="v")
        yT_sb = y_pool.tile([128, 2, L], dt_y, tag="y")
        v5 = v_sb.rearrange("p t (hp par c) -> p t hp par c", hp=2, par=2)
        # v_aug ones/zeros scaffold: one clean [128,512] DMA per token block
        # (the v-value regions are overwritten by the projection copies)
        for tb in range(16):
            nc.scalar.dma_start(v_sb[:, tb, :], vconst)

        # persistent reciprocal-row tiles (zeroed once; only one row is ever
        # rewritten, the bc selector matmul ignores every other row but they
        # must not hold NaN garbage since 0*NaN = NaN)
        rrE = const_pool.tile([128, 512], f32r, tag="rrE")
        nc.scalar.dma_start(rrE, zpad)
        rrO = const_pool.tile([128, 512], f32r, tag="rrO")
        nc.scalar.dma_start(rrO, zpad)

        x_pool = ctx.enter_context(tc.tile_pool(name="xp", bufs=1))
        x_sb = x_pool.tile([128, 8, L], dt_x, tag="x")
        e_pool = ctx.enter_context(tc.tile_pool(name="ep", bufs=6))
        rb_pool = ctx.enter_context(tc.tile_pool(name="rb", bufs=3))
        ob_pool = ctx.enter_context(tc.tile_pool(name="ob", bufs=4))

        wq_sb = w_pool.tile([128, 8, DLOC], dt_qk, tag="wq")
        wk_sb = w_pool.tile([128, 8, DLOC], dt_qk, tag="wk")
        wv_sb = w_pool.tile([128, 8, DLOC], dt_x, tag="wv")
        for w_sb, w_dram in ((wq_sb, wqT), (wk_sb, wkT), (wv_sb, wvT)):
            for c in range(8):
                nc.sync.dma_start(w_sb[:, c, :], w_dram[128 * c : 128 * c + 128, :])
        wo_sb = w_pool.tile([128, 2, E], dt_y, tag="wo")
        for c in range(2):
            nc.sync.dma_start(wo_sb[:, c, :], woM[128 * c : 128 * c + 128, :])

        # Deferred-work queue: the normalize chain (reciprocal -> selector
        # matmul -> mul) and the out-projection of a finished query tile are
        # emitted one iteration later, so their PE instructions never
        # head-of-line-block the PE stream (each such stall re-cools the HAM
        # clock gate and halves matmul throughput).
        laters = []

        def norm_closure(h, i, ya):
            blk, par = h // 2, h % 2
            p0 = 64 * par

            def norm():
                if par == 0:
                    rr, dr, sel = rrE, 64, bcsel_sb[:, 0:64]
                else:
                    rr, dr, sel = rrO, 0, bcsel_sb[:, 64:128]
                # stage the raw denominator row (selector matmul rhs must be
                # SBUF), broadcast it across partitions 0..63 with the
                # selector matmul, THEN take the reciprocal on 64 lanes in
                # parallel (a [1,512] single-lane reciprocal costs 3.4us).
                nc.vector.tensor_copy(rr[dr : dr + 1, :], ya[dr : dr + 1, :])
                bc = pp_psum.tile([128, 512], f32, tag="pp")
                nc.tensor.matmul(bc[0:64, :], sel, rr, start=True, stop=True)
                rbs = rb_pool.tile([128, 512], f32, tag="rbs")
                nc.vector.tensor_copy(rbs[0:64, :], bc[0:64, :])
                rb = rb_pool.tile([128, 512], f32, tag="rb")
                # (custom DVE op quirks on HW: PSUM reads and nonzero base
                # partition both produce garbage - keep it SBUF at offset 0)
                nc.vector.reciprocal_approx_fast(
                    out=rb[0:64, :], in_=rbs[0:64, :]
                )
                if par != 0:
                    # odd heads consume the reciprocal on partitions 64..127;
                    # only a DMA can cross partitions (gpsimd queue is idle
                    # during attention)
                    nc.gpsimd.dma_start(rb[64:128, :], rb[0:64, :])
                nc.vector.tensor_mul(
                    yT_sb[p0 : p0 + 64, blk, 512 * i : 512 * i + 512],
                    ya[p0 : p0 + 64, :],
                    rb[p0 : p0 + 64, :],
                )

            return norm

        def outproj_closure(i):
            def outproj():
                for tb in range(4 * i, 4 * i + 4):
                    for et in range(2):
                        ps = pp_psum.tile([128, 512], f32, tag="pp")
                        for c in range(2):
                            nc.tensor.matmul(
                                ps,
                                yT_sb[:, c, 128 * tb : 128 * tb + 128],
                                wo_sb[:, c, 512 * et : 512 * et + 512],
                                start=(c == 0),
                                stop=(c == 1),
                            )
                        ob = ob_pool.tile([128, 512], f32, tag="o")
                        if et == 0:
                            nc.vector.tensor_copy(ob, ps)
                        else:
                            nc.scalar.copy(ob, ps)
                        nc.sync.dma_start(
                            outp[128 * tb : 128 * tb + 128, 512 * et : 512 * et + 512],
                            ob,
                        )

            return outproj

        # ---- main loop ----
        # For each 512-token tile: load x + project, then run attention for
        # the same query tile (its deps are exactly tiles <= i). The NEXT
        # tile's projections are interleaved between attention heads so the
        # scalar engine's softmax exps never starve behind a monolithic
        # projection block.
        def emit_xdma(i):
            tsl = slice(512 * i, 512 * i + 512)
            for c in range(8):
                nc.gpsimd.dma_start(x_sb[:, c, tsl], xT[128 * c : 128 * c + 128, tsl])

        def emit_proj_part(i, part):
            tsl = slice(512 * i, 512 * i + 512)
            if part in (0, 1):
                # qT/kT [256, 2048] = W_loc @ x.T
                w_sb, dst = ((wq_sb, q_sb), (wk_sb, k_sb))[part]
                for blk in range(2):
                    ps = pp_psum.tile([128, 512], f32, tag="pp")
                    for c in range(8):
                        nc.tensor.matmul(
                            ps,
                            w_sb[:, c, 128 * blk : 128 * blk + 128],
                            x_sb[:, c, tsl],
                            start=(c == 0),
                            stop=(c == 7),
                        )
                    nc.vector.tensor_copy(dst[:, blk, tsl], ps)
            else:
                # v natural [t, dims]: lhsT = xT chunk (stationary)
                for tb in range(4 * i + 2 * (part - 2), 4 * i + 2 * (part - 2) + 2):
                    ps = pp_psum.tile([128, 256], f32, tag="pp")
                    for c in range(8):
                        nc.tensor.matmul(
                            ps,
                            x_sb[:, c, 128 * tb : 128 * tb + 128],
                            wv_sb[:, c, :],
                            start=(c == 0),
                            stop=(c == 7),
                        )
                    psr = ps.rearrange("p (hp par c) -> p hp par c", hp=2, par=2)
                    nc.vector.tensor_copy(v5[:, tb, :, 0, 0:D], psr[:, :, 0, :])
                    nc.vector.tensor_copy(v5[:, tb, :, 1, D:128], psr[:, :, 1, :])

        emit_xdma(0)
        for part in range(4):
            emit_proj_part(0, part)
        for i in range(4):
            if i < 3:
                emit_xdma(i + 1)
            # attention for query tile i
            nch = 4 * i + 4  # causal: key chunks 0..4i+3
            for h in range(4):
                blk, par = h // 2, h % 2
                p0 = 64 * par  # partition offset of this head's qT/kT rows
                ya = ya_psum.tile([128, 512], f32, tag="ya")
                for jp in range(nch // 2):  # key chunks processed in pairs
                    j0, j1 = 2 * jp, 2 * jp + 1
                    partial = jp >= 2 * i  # both chunks of a pair agree
                    ps2 = sc_psum.tile([128, 1024], f32, tag="sc")
                    # within a partial block at key offset o, only queries
                    # t >= o can attend - stream 512-o rows instead of 512
                    offs = [
                        max(0, 128 * j - 512 * i) if partial else 0
                        for j in (j0, j1)
                    ]
                    for hi, j in ((0, j0), (1, j1)):
                        o = offs[hi]
                        # scoresT block [128 keys, 512-o queries]
                        nc.tensor.matmul(
                            ps2[:, 512 * hi + o : 512 * hi + 512],
                            k_sb[p0 : p0 + 64, blk, 128 * j : 128 * j + 128],
                            q_sb[p0 : p0 + 64, blk, 512 * i + o : 512 * i + 512],
                            start=True,
                            stop=True,
                        )
                    e = e_pool.tile([128, 1024], dt_v, tag="e")
                    if not partial:
                        nc.scalar.activation(e, ps2, Exp, scale=0.125)
                    else:
                        s = jp - 2 * i
                        for hi in range(2):
                            o = offs[hi]
                            lo, hs = 512 * hi + o, 512 - o
                            nc.scalar.activation(
                                e[:, lo : lo + hs], ps2[:, lo : lo + hs],
                                Exp, scale=0.125,
                            )
                            # causal mask: zero the exp of future keys
                            # (gpsimd is idle during attention; keeps the DVE
                            # queue off the exp->attnV chain)
                            nc.gpsimd.tensor_mul(
                                e[:, lo : lo + hs],
                                e[:, lo : lo + hs],
                                pm_sb[:, 1024 * s + lo : 1024 * s + lo + hs],
                            )
                    # attn@V (+ softmax denominator via the ones column)
                    for hi, j in ((0, j0), (1, j1)):
                        o = offs[hi]
                        nc.tensor.matmul(
                            ya[:, o:512],
                            v_sb[:, j, 128 * h : 128 * h + 128],
                            e[:, 512 * hi + o : 512 * hi + 512],
                            start=(j == 0),
                            stop=(j == nch - 1),
                            skip_group_check=True,
                        )
                newly = [norm_closure(h, i, ya)]
                if h == 3:
                    newly.append(outproj_closure(i))
                for fn in laters:
                    fn()
                laters = newly
                if i < 3:
                    emit_proj_part(i + 1, h)
        for fn in laters:
            fn()

    nc.compile()
    return nc


def _get_program():
    if "nc" not in _CACHE:
        _CACHE["nc"] = _build_program()
    return _CACHE["nc"]


def _host_consts():
    import ml_dtypes

    dts = dict(bf16=("bf16", "bf16"), mixed=("f32", "bf16"), f32r=("f32", "f32"))
    qk_t, v_t = dts[KOMP]
    bf16 = ml_dtypes.bfloat16

    def conv(a, kind):
        return a.astype(bf16) if kind == "bf16" else a.astype(np.float32)

    # pairmask[s]: [mask(o=256s) | mask(o=256s+128)], mask(o)[p,t] = 0 iff
    # key (p+o) > query (t) else 1
    pairmask = np.zeros((128, 2048), dtype=np.float32)
    p = np.arange(128)[:, None]
    t = np.arange(512)[None, :]
    for s in range(2):
        for hi in range(2):
            o = 256 * s + 128 * hi
            pairmask[:, 1024 * s + 512 * hi : 1024 * s + 512 * hi + 512] = (
                t >= p + o
            ).astype(np.float32)
    vconst = np.zeros((128, 512), dtype=np.float32)
    for h in range(4):
        if h % 2 == 0:
            vconst[:, 128 * h + 64] = 1.0   # even head: ones col at 64
        else:
            vconst[:, 128 * h + 0] = 1.0    # odd head: ones col at 0
    bcsel = np.zeros((128, 128), dtype=np.float32)
    bcsel[64, 0:64] = 1.0   # even heads: extract rr row 64 -> out parts 0..63
    bcsel[0, 64:128] = 1.0  # odd heads: extract rr row 0 -> out parts 0..63
    return {
        "pairmask": conv(pairmask, v_t),
        "zpad": np.zeros((128, 512), dtype=np.float32),
        "vconst": conv(vconst, v_t),
        "bcsel": bcsel,
    }


def _enable_trace_support():
    """Best-effort: make trace=True work in this container (NTFF hook shim +
    disable artifact upload). No-op if anything is missing."""
    import sys
    import types

    try:
        import concourse.bass_utils as bu

        bu.upload_artifacts = lambda tmpdir: tmpdir
        try:
            from antenv.axon_hooks import get_axon_ntff_profile_hook  # noqa: F401

            return True
        except ImportError:
            pass
        import antenv
        from trn_agent_boot.trn_boot import _ntff_profile_via_ctypes

        hook = _ntff_profile_via_ctypes("/opt/axon/libaxon_pjrt.so")
        mod = types.ModuleType("antenv.axon_hooks")
        state = {"hook": hook}
        mod.get_axon_ntff_profile_hook = lambda: state["hook"]
        mod.set_axon_ntff_profile_hook = lambda h: state.__setitem__("hook", h)
        sys.modules["antenv.axon_hooks"] = mod
        antenv.axon_hooks = mod
        return hook is not None
    except Exception:
        return False


def kernel(x, attention_mask, Wq, Wk, Wv, Wo, bo):
    global LAST_RESULTS
    import ml_dtypes
    from concourse.bass_utils import run_bass_kernel_spmd

    x = np.asarray(x, dtype=np.float32)
    attention_mask = np.asarray(attention_mask, dtype=np.float32)
    Wq = np.asarray(Wq, dtype=np.float32)
    Wk = np.asarray(Wk, dtype=np.float32)
    Wv = np.asarray(Wv, dtype=np.float32)
    Wo = np.asarray(Wo, dtype=np.float32)
    bo = np.asarray(bo, dtype=np.float32)

    nc = _get_program()

    dts = dict(bf16=("bf16", "bf16"), mixed=("f32", "bf16"), f32r=("f32", "f32"))
    qk_t, v_t = dts[KOMP]
    bf16 = ml_dtypes.bfloat16

    def conv(a, kind):
        a = np.ascontiguousarray(a)
        return a.astype(bf16) if kind == "bf16" else a

    # host-side shard prep
    xm = x * attention_mask[:, :, None]
    xTs = [conv(xm[b].T, qk_t) for b in range(B)]
    consts = _host_consts()

    in_maps = []
    for core in range(8):
        b, g = divmod(core, 4)
        sl = slice(DLOC * g, DLOC * g + DLOC)
        in_maps.append(
            {
                "xT": xTs[b],
                "wqT": conv(Wq[sl].T, qk_t),
                "wkT": conv(Wk[sl].T, qk_t),
                "wvT": conv(Wv[sl].T, qk_t),
                "woM": conv(Wo[:, sl].T, v_t),
                **consts,
            }
        )

    trace = bool(int(os.environ.get("KERNEL_TRACE", "0")))
    if trace:
        trace = _enable_trace_support()
    res = run_bass_kernel_spmd(nc, in_maps, core_ids=list(range(8)), trace=trace)
    LAST_RESULTS = res

    out = np.zeros((B, L, E), dtype=np.float32)
    for core in range(8):
        out[core // 4] += res.results[core]["outp"]
    out += bo
    return out


# revision 43
# speedup vs baseline: 1.0246x; 1.0246x over previous
"""Trainium2 Bass kernel for EvoAttn (B=2, L=2048, E=1024, H=16, D=64, causal,
multiplicative attention mask on q/k/v, fp32).

Sharding: batch*heads across 8 cores. Core c handles batch c//4, heads
[4*(c%4), 4*(c%4)+4). Each core computes its 4 heads' q/k/v projections
(column-parallel), full local attention, and a partial out-projection
(row-parallel). Partials are summed on the host (unshard) and bias added.

Layout notes (per core):
  xT   [1024, 2048]  = (x[b] * mask[b][:,None]).T           (host-prepared)
  wqT/wkT/wvT [1024, 256] = W[rows].T for the 256 local dims
  woM  [256, 1024]   = Wo[:, local].T
  qT/kT [256, 2048] in two partition blocks; head h at partitions 64*(h%2)..+63
  of block h//2. Scores are computed transposed (keys on partitions, queries on
  the free dim) so softmax needs no transposes: V is augmented with a ones
  column so the attn@V matmul also emits the softmax denominator. The V
  augmentation layout depends on head parity so each head's output lands on
  the partition half its yT slot needs (DVE ops cannot cross partitions):
    even head: [v(64) | ones(1) | zeros(63)] -> y rows 0..63,  denom row 64
    odd head:  [ones(1) | zeros(63) | v(64)] -> y rows 64..127, denom row 0
  Causal masking = skip fully-masked key blocks + add a -1e30 staircase mask
  (via an identity matmul) into partially-masked score blocks in PSUM.

Compute dtype config (KOMP env): "bf16" everything bf16; "mixed" keeps the
q/k/score path in float32r (tf32-like) for accuracy and uses bf16 for the
attn@V / out-proj bulk; "f32r" everything float32r.
"""

import os
import numpy as np

B, L, E, H, D = 2, 2048, 1024, 16, 64
DLOC = E // 4          # local out dims per core (4 heads * 64)
NEG = -1.0e30
KOMP = os.environ.get("KOMP", "bf16")

_CACHE = {}
LAST_RESULTS = None


def _dtype_cfg(mybir):
    f32r, bf16 = mybir.dt.float32r, mybir.dt.bfloat16
    if KOMP == "bf16":
        return dict(qk=bf16, v=bf16, y=bf16, mask=bf16)
    if KOMP == "mixed":
        return dict(qk=f32r, v=bf16, y=bf16, mask=f32r)
    return dict(qk=f32r, v=f32r, y=f32r, mask=f32r)


def _build_program():
    from contextlib import ExitStack

    import concourse.bacc as bacc
    import concourse.mybir as mybir
    import concourse.tile as tile

    f32 = mybir.dt.float32
    f32r = mybir.dt.float32r
    Exp = mybir.ActivationFunctionType.Exp
    dts = _dtype_cfg(mybir)
    dt_qk, dt_v, dt_y, dt_mask = dts["qk"], dts["v"], dts["y"], dts["mask"]
    # x and wq/wk feed the q/k path; wv feeds v (paired with x in its matmul,
    # so it must match x's dtype)
    dt_x = dt_qk

    nc = bacc.Bacc("TRN2", target_bir_lowering=False, debug=False, num_devices=8)

    def dram_in(name, shape, dt):
        # float32r inputs are declared f32 and bitcast (same bits); bf16
        # inputs are host-converted
        dd = f32 if dt in (f32, f32r) else dt
        ap = nc.dram_tensor(name, shape, dd, kind="ExternalInput").ap()
        return ap.bitcast(dt) if dt == f32r else ap

    xT = dram_in("xT", [E, L], dt_x)
    wqT = dram_in("wqT", [E, DLOC], dt_qk)
    wkT = dram_in("wkT", [E, DLOC], dt_qk)
    wvT = dram_in("wvT", [E, DLOC], dt_x)
    woM = dram_in("woM", [DLOC, E], dt_y)
    # 0/1 causal masks for the two partial chunk-pair offsets (see phase 2)
    pairmask = dram_in("pairmask", [128, 2048], dt_v)
    zpad = dram_in("zpad", [128, 512], f32r)
    vconst = dram_in("vconst", [128, 512], dt_v)
    # bc selector patterns: [:,0:64] row-64 extractor, [:,64:192] row-0 ->
    # output partitions 64..127 extractor
    bcsel = dram_in("bcsel", [128, 128], f32r)
    outp = nc.dram_tensor("outp", [L, E], f32, kind="ExternalOutput").ap()

    with (
        tile.TileContext(nc) as tc,
        ExitStack() as ctx,
        nc.allow_low_precision(reason="float32r/bf16 matmul inputs"),
    ):
        const_pool = ctx.enter_context(tc.tile_pool(name="const", bufs=1))
        w_pool = ctx.enter_context(tc.tile_pool(name="wp", bufs=1))
        qk_pool = ctx.enter_context(tc.tile_pool(name="qk", bufs=1))
        v_pool = ctx.enter_context(tc.tile_pool(name="vp", bufs=1))
        y_pool = ctx.enter_context(tc.tile_pool(name="yp", bufs=1))
        pp_psum = ctx.enter_context(tc.tile_pool(name="pp", bufs=2, space="PSUM"))
        sc_psum = ctx.enter_context(tc.tile_pool(name="sc", bufs=2, space="PSUM"))
        ya_psum = ctx.enter_context(tc.tile_pool(name="ya", bufs=2, space="PSUM"))

        # ---- constants (small loads, issued on scalar engine) ----
        pm_sb = const_pool.tile([128, 2048], dt_v, tag="pm")
        nc.scalar.dma_start(pm_sb, pairmask)
        bcsel_sb = const_pool.tile([128, 128], f32r, tag="bcsel")
        nc.scalar.dma_start(bcsel_sb, bcsel)

        q_sb = qk_pool.tile([128, 2, L], dt_qk, tag="q")
        k_sb = qk_pool.tile([128, 2, L], dt_qk, tag="k")
        # v augmented per head parity: 128 cols per head (see module docstring)
        v_sb = v_pool.tile([128, 16, 4 * 128], dt_v, tag="v")
        yT_sb = y_pool.tile([128, 2, L], dt_y, tag="y")
        v5 = v_sb.rearrange("p t (hp par c) -> p t hp par c", hp=2, par=2)
        # v_aug ones/zeros scaffold: one clean [128,512] DMA per token block
        # (the v-value regions are overwritten by the projection copies)
        for tb in range(16):
            nc.scalar.dma_start(v_sb[:, tb, :], vconst)

        # persistent reciprocal-row tiles (zeroed once; only one row is ever
        # rewritten, the bc selector matmul ignores every other row but they
        # must not hold NaN garbage since 0*NaN = NaN)
        rrE = const_pool.tile([128, 512], f32r, tag="rrE")
        nc.scalar.dma_start(rrE, zpad)
        rrO = const_pool.tile([128, 512], f32r, tag="rrO")
        nc.scalar.dma_start(rrO, zpad)

        x_pool = ctx.enter_context(tc.tile_pool(name="xp", bufs=1))
        x_sb = x_pool.tile([128, 8, L], dt_x, tag="x")
        e_pool = ctx.enter_context(tc.tile_pool(name="ep", bufs=8))
        rb_pool = ctx.enter_context(tc.tile_pool(name="rb", bufs=3))
        ob_pool = ctx.enter_context(tc.tile_pool(name="ob", bufs=4))

        wq_sb = w_pool.tile([128, 8, DLOC], dt_qk, tag="wq")
        wk_sb = w_pool.tile([128, 8, DLOC], dt_qk, tag="wk")
        wv_sb = w_pool.tile([128, 8, DLOC], dt_x, tag="wv")
        for w_sb, w_dram in ((wq_sb, wqT), (wk_sb, wkT), (wv_sb, wvT)):
            for c in range(8):
                nc.sync.dma_start(w_sb[:, c, :], w_dram[128 * c : 128 * c + 128, :])
        wo_sb = w_pool.tile([128, 2, E], dt_y, tag="wo")
        for c in range(2):
            nc.sync.dma_start(wo_sb[:, c, :], woM[128 * c : 128 * c + 128, :])

        # Deferred-work queue: the normalize chain (reciprocal -> selector
        # matmul -> mul) and the out-projection of a finished query tile are
        # emitted one iteration later, so their PE instructions never
        # head-of-line-block the PE stream (each such stall re-cools the HAM
        # clock gate and halves matmul throughput).
        laters = []

        def norm_closure(h, i, ya):
            blk, par = h // 2, h % 2
            p0 = 64 * par

            def norm():
                if par == 0:
                    rr, dr, sel = rrE, 64, bcsel_sb[:, 0:64]
                else:
                    rr, dr, sel = rrO, 0, bcsel_sb[:, 64:128]
                # stage the raw denominator row (selector matmul rhs must be
                # SBUF), broadcast it across partitions 0..63 with the
                # selector matmul, THEN take the reciprocal on 64 lanes in
                # parallel (a [1,512] single-lane reciprocal costs 3.4us).
                nc.vector.tensor_copy(rr[dr : dr + 1, :], ya[dr : dr + 1, :])
                bc = pp_psum.tile([128, 512], f32, tag="pp")
                nc.tensor.matmul(bc[0:64, :], sel, rr, start=True, stop=True)
                rbs = rb_pool.tile([128, 512], f32, tag="rbs")
                # ACT copy: lands in the scalar engine's qtile-boundary idle
                # gaps, keeping the DVE queue short for the mask multiplies
                nc.scalar.copy(rbs[0:64, :], bc[0:64, :])
                rb = rb_pool.tile([128, 512], f32, tag="rb")
                # (custom DVE op quirks on HW: PSUM reads and nonzero base
                # partition both produce garbage - keep it SBUF at offset 0)
                nc.vector.reciprocal_approx_fast(
                    out=rb[0:64, :], in_=rbs[0:64, :]
                )
                if par != 0:
                    # odd heads consume the reciprocal on partitions 64..127;
                    # only a DMA can cross partitions (gpsimd queue is idle
                    # during attention)
                    nc.gpsimd.dma_start(rb[64:128, :], rb[0:64, :])
                nc.vector.tensor_mul(
                    yT_sb[p0 : p0 + 64, blk, 512 * i : 512 * i + 512],
                    ya[p0 : p0 + 64, :],
                    rb[p0 : p0 + 64, :],
                )

            return norm

        def outproj_closure(i):
            def outproj():
                for tb in range(4 * i, 4 * i + 4):
                    for et in range(2):
                        ps = pp_psum.tile([128, 512], f32, tag="pp")
                        for c in range(2):
                            nc.tensor.matmul(
                                ps,
                                yT_sb[:, c, 128 * tb : 128 * tb + 128],
                                wo_sb[:, c, 512 * et : 512 * et + 512],
                                start=(c == 0),
                                stop=(c == 1),
                            )
                        ob = ob_pool.tile([128, 512], f32, tag="o")
                        if et == 0:
                            nc.vector.tensor_copy(ob, ps)
                        else:
                            nc.scalar.copy(ob, ps)
                        nc.sync.dma_start(
                            outp[128 * tb : 128 * tb + 128, 512 * et : 512 * et + 512],
                            ob,
                        )

            return outproj

        # ---- main loop ----
        # For each 512-token tile: load x + project, then run attention for
        # the same query tile (its deps are exactly tiles <= i). The NEXT
        # tile's projections are interleaved between attention heads so the
        # scalar engine's softmax exps never starve behind a monolithic
        # projection block.
        def emit_xdma(i):
            tsl = slice(512 * i, 512 * i + 512)
            for c in range(8):
                nc.gpsimd.dma_start(x_sb[:, c, tsl], xT[128 * c : 128 * c + 128, tsl])

        def emit_proj_part(i, part):
            tsl = slice(512 * i, 512 * i + 512)
            if part in (0, 1):
                # qT/kT [256, 2048] = W_loc @ x.T
                w_sb, dst = ((wq_sb, q_sb), (wk_sb, k_sb))[part]
                for blk in range(2):
                    ps = pp_psum.tile([128, 512], f32, tag="pp")
                    for c in range(8):
                        nc.tensor.matmul(
                            ps,
                            w_sb[:, c, 128 * blk : 128 * blk + 128],
                            x_sb[:, c, tsl],
                            start=(c == 0),
                            stop=(c == 7),
                        )
                    nc.vector.tensor_copy(dst[:, blk, tsl], ps)
            else:
                # v natural [t, dims]: lhsT = xT chunk (stationary)
                for tb in range(4 * i + 2 * (part - 2), 4 * i + 2 * (part - 2) + 2):
                    ps = pp_psum.tile([128, 256], f32, tag="pp")
                    for c in range(8):
                        nc.tensor.matmul(
                            ps,
                            x_sb[:, c, 128 * tb : 128 * tb + 128],
                            wv_sb[:, c, :],
                            start=(c == 0),
                            stop=(c == 7),
                        )
                    psr = ps.rearrange("p (hp par c) -> p hp par c", hp=2, par=2)
                    nc.vector.tensor_copy(v5[:, tb, :, 0, 0:D], psr[:, :, 0, :])
                    nc.vector.tensor_copy(v5[:, tb, :, 1, D:128], psr[:, :, 1, :])

        emit_xdma(0)
        for part in range(4):
            emit_proj_part(0, part)
        for i in range(4):
            if i < 3:
                emit_xdma(i + 1)
            # attention for query tile i
            nch = 4 * i + 4  # causal: key chunks 0..4i+3
            for h in range(4):
                blk, par = h // 2, h % 2
                p0 = 64 * par  # partition offset of this head's qT/kT rows
                ya = ya_psum.tile([128, 512], f32, tag="ya")
                for jp in range(nch // 2):  # key chunks processed in pairs
                    j0, j1 = 2 * jp, 2 * jp + 1
                    partial = jp >= 2 * i  # both chunks of a pair agree
                    ps2 = sc_psum.tile([128, 1024], f32, tag="sc")
                    # within a partial block at key offset o, only queries
                    # t >= o can attend - stream 512-o rows instead of 512
                    offs = [
                        max(0, 128 * j - 512 * i) if partial else 0
                        for j in (j0, j1)
                    ]
                    for hi, j in ((0, j0), (1, j1)):
                        o = offs[hi]
                        # scoresT block [128 keys, 512-o queries]
                        nc.tensor.matmul(
                            ps2[:, 512 * hi + o : 512 * hi + 512],
                            k_sb[p0 : p0 + 64, blk, 128 * j : 128 * j + 128],
                            q_sb[p0 : p0 + 64, blk, 512 * i + o : 512 * i + 512],
                            start=True,
                            stop=True,
                        )
                    e = e_pool.tile([128, 1024], dt_v, tag="e")
                    if not partial:
                        nc.scalar.activation(e, ps2, Exp, scale=0.125)
                    else:
                        s = jp - 2 * i
                        for hi in range(2):
                            o = offs[hi]
                            lo, hs = 512 * hi + o, 512 - o
                            nc.scalar.activation(
                                e[:, lo : lo + hs], ps2[:, lo : lo + hs],
                                Exp, scale=0.125,
                            )
                            # causal mask: zero the exp of future keys
                            nc.vector.tensor_mul(
                                e[:, lo : lo + hs],
                                e[:, lo : lo + hs],
                                pm_sb[:, 1024 * s + lo : 1024 * s + lo + hs],
                            )
                    # attn@V (+ softmax denominator via the ones column)
                    for hi, j in ((0, j0), (1, j1)):
                        o = offs[hi]
                        nc.tensor.matmul(
                            ya[:, o:512],
                            v_sb[:, j, 128 * h : 128 * h + 128],
                            e[:, 512 * hi + o : 512 * hi + 512],
                            start=(j == 0),
                            stop=(j == nch - 1),
                            skip_group_check=True,
                        )
                newly = [norm_closure(h, i, ya)]
                if h == 3:
                    newly.append(outproj_closure(i))
                for fn in laters:
                    fn()
                laters = newly
                if i < 3:
                    emit_proj_part(i + 1, h)
        for fn in laters:
            fn()

    nc.compile()
    return nc


def _get_program():
    if "nc" not in _CACHE:
        _CACHE["nc"] = _build_program()
    return _CACHE["nc"]


def _host_consts():
    import ml_dtypes

    dts = dict(bf16=("bf16", "bf16"), mixed=("f32", "bf16"), f32r=("f32", "f32"))
    qk_t, v_t = dts[KOMP]
    bf16 = ml_dtypes.bfloat16

    def conv(a, kind):
        return a.astype(bf16) if kind == "bf16" else a.astype(np.float32)

    # pairmask[s]: [mask(o=256s) | mask(o=256s+128)], mask(o)[p,t] = 0 iff
    # key (p+o) > query (t) else 1
    pairmask = np.zeros((128, 2048), dtype=np.float32)
    p = np.arange(128)[:, None]
    t = np.arange(512)[None, :]
    for s in range(2):
        for hi in range(2):
            o = 256 * s + 128 * hi
            pairmask[:, 1024 * s + 512 * hi : 1024 * s + 512 * hi + 512] = (
                t >= p + o
            ).astype(np.float32)
    vconst = np.zeros((128, 512), dtype=np.float32)
    for h in range(4):
        if h % 2 == 0:
            vconst[:, 128 * h + 64] = 1.0   # even head: ones col at 64
        else:
            vconst[:, 128 * h + 0] = 1.0    # odd head: ones col at 0
    bcsel = np.zeros((128, 128), dtype=np.float32)
    bcsel[64, 0:64] = 1.0   # even heads: extract rr row 64 -> out parts 0..63
    bcsel[0, 64:128] = 1.0  # odd heads: extract rr row 0 -> out parts 0..63
    return {
        "pairmask": conv(pairmask, v_t),
        "zpad": np.zeros((128, 512), dtype=np.float32),
        "vconst": conv(vconst, v_t),
        "bcsel": bcsel,
    }


def _enable_trace_support():
    """Best-effort: make trace=True work in this container (NTFF hook shim +
    disable artifact upload). No-op if anything is missing."""
    import sys
    import types

    try:
        import concourse.bass_utils as bu

        bu.upload_artifacts = lambda tmpdir: tmpdir
        try:
            from antenv.axon_hooks import get_axon_ntff_profile_hook  # noqa: F401

            return True
        except ImportError:
            pass
        import antenv
        from trn_agent_boot.trn_boot import _ntff_profile_via_ctypes

        hook = _ntff_profile_via_ctypes("/opt/axon/libaxon_pjrt.so")
        mod = types.ModuleType("antenv.axon_hooks")
        state = {"hook": hook}
        mod.get_axon_ntff_profile_hook = lambda: state["hook"]
        mod.set_axon_ntff_profile_hook = lambda h: state.__setitem__("hook", h)
        sys.modules["antenv.axon_hooks"] = mod
        antenv.axon_hooks = mod
        return hook is not None
    except Exception:
        return False


def kernel(x, attention_mask, Wq, Wk, Wv, Wo, bo):
    global LAST_RESULTS
    import ml_dtypes
    from concourse.bass_utils import run_bass_kernel_spmd

    x = np.asarray(x, dtype=np.float32)
    attention_mask = np.asarray(attention_mask, dtype=np.float32)
    Wq = np.asarray(Wq, dtype=np.float32)
    Wk = np.asarray(Wk, dtype=np.float32)
    Wv = np.asarray(Wv, dtype=np.float32)
    Wo = np.asarray(Wo, dtype=np.float32)
    bo = np.asarray(bo, dtype=np.float32)

    nc = _get_program()

    dts = dict(bf16=("bf16", "bf16"), mixed=("f32", "bf16"), f32r=("f32", "f32"))
    qk_t, v_t = dts[KOMP]
    bf16 = ml_dtypes.bfloat16

    def conv(a, kind):
        a = np.ascontiguousarray(a)
        return a.astype(bf16) if kind == "bf16" else a

    # host-side shard prep
    xm = x * attention_mask[:, :, None]
    xTs = [conv(xm[b].T, qk_t) for b in range(B)]
    consts = _host_consts()

    in_maps = []
    for core in range(8):
        b, g = divmod(core, 4)
        sl = slice(DLOC * g, DLOC * g + DLOC)
        in_maps.append(
            {
                "xT": xTs[b],
                "wqT": conv(Wq[sl].T, qk_t),
                "wkT": conv(Wk[sl].T, qk_t),
                "wvT": conv(Wv[sl].T, qk_t),
                "woM": conv(Wo[:, sl].T, v_t),
                **consts,
            }
        )

    trace = bool(int(os.environ.get("KERNEL_TRACE", "0")))
    if trace:
        trace = _enable_trace_support()
    res = run_bass_kernel_spmd(nc, in_maps, core_ids=list(range(8)), trace=trace)
    LAST_RESULTS = res

    out = np.zeros((B, L, E), dtype=np.float32)
    for core in range(8):
        out[core // 4] += res.results[core]["outp"]
    out += bo
    return out
